# revision 29
# baseline (speedup 1.0000x reference)
"""Joseph 3D projector on 8 TRN2 NeuronCores — banded-matmul version.

Formulation: for each angle a, out[u, v] = DT * sum_p M_a[p, u] * volT[p, v]
where p = y*128 + x and volT[p, v] = vol[0,0,x,y,v] (the reference's
z-interpolation is an exact identity for this geometry). M_a is ~1.5% dense:
for a fixed contraction slab (a y-line or x-line of the volume) the nonzero
u's lie in a narrow window of width ~128*min(|sin|,|cos|)+4. So instead of
streaming dense M (4MB/angle bf16, DMA-bound at ~212us), we stream only the
per-slab windows in fp8-e3m4 and issue one small matmul per (slab, angle)
accumulating at the window's column offset in PSUM (per-element has_written
semantics: first-touch overwrites, later touches accumulate).

Per angle the contraction axis is chosen as the driving axis (contract over
x with y-slabs when |sin|<=|cos|, else over y with x-slabs), against one of
two SBUF-resident volume layouts (volA=[x,(y,v)], volB=[y,(x,v)], bf16).

Sharding: 15 angle-slots; slot j on core c holds angle k = 8j + c, so all 8
cores share one SPMD program whose per-slot window geometry (width, per-slab
column offsets) covers the slot's 8 consecutive angles.

The stationary operand (the vol slab) is shared by all of a slab's matmuls;
redundant LDWEIGHTS are elided via InstMatmult.ldweights=False.
"""
import numpy as np
import ml_dtypes

D = H = W = 128
V = U = 128
A = 120
S = 128
NCORES = 8
NSLOT = A // NCORES  # 15
T = 0.5 * float(np.sqrt(((W - 1) * 1.0) ** 2 + ((H - 1) * 1.0) ** 2))
DT = 2.0 * T / S

ELIDE_LDW = False      # InstMatmult.ldweights=False proved inert on HW
MGROUP = 16            # slabs per M dma group
PREFETCH = 2           # dma groups emitted ahead of compute


def _build_M(cos_t, sin_t):
    """Dense M[p=(y*W+x), u] float32 for one angle."""
    u_phys = np.arange(U, dtype=np.float64) - (U - 1) / 2.0
    t = -T + (np.arange(S, dtype=np.float64) + 0.5) * DT
    x_idx = (-u_phys[None, :] * sin_t + t[:, None] * cos_t) + (W - 1) / 2.0
    y_idx = (u_phys[None, :] * cos_t + t[:, None] * sin_t) + (H - 1) / 2.0
    x0 = np.floor(x_idx).astype(np.int64)
    y0 = np.floor(y_idx).astype(np.int64)
    wx = x_idx - x0
    wy = y_idx - y0
    Mflat = np.zeros(H * W * U, np.float32)
    uu = np.broadcast_to(np.arange(U, dtype=np.int64)[None, :], (S, U))
    for dy, dx in ((0, 0), (0, 1), (1, 0), (1, 1)):
        yi = y0 + dy
        xi = x0 + dx
        w = (wy if dy else 1 - wy) * (wx if dx else 1 - wx)
        valid = (xi >= 0) & (xi <= W - 1) & (yi >= 0) & (yi <= H - 1)
        p = np.clip(yi, 0, H - 1) * W + np.clip(xi, 0, W - 1)
        flat = (p * U + uu)[valid]
        Mflat += np.bincount(flat, weights=w[valid].astype(np.float64),
                             minlength=H * W * U).astype(np.float32)
    return Mflat.reshape(H * W, U)


def _schedule(angles):
    """Slot geometry shared by all cores: per slot j (angles 8j..8j+7):
    contraction axis, window width, per-slab window offsets."""
    Ms = np.stack([_build_M(np.cos(np.float64(a)), np.sin(np.float64(a)))
                   for a in angles])              # [A, H*W, U]
    axes, widths, offs = [], [], []
    for j in range(NSLOT):
        ks = list(range(NCORES * j, NCORES * j + NCORES))
        th = [float(angles[k]) for k in ks]
        s_m = np.mean([abs(np.sin(t)) for t in th])
        c_m = np.mean([abs(np.cos(t)) for t in th])
        ax = 0 if s_m <= c_m else 1
        lo = np.full(S, U, np.int64)
        hi = np.full(S, -1, np.int64)
        for k in ks:
            Mr = Ms[k].reshape(H, W, U)
            sl = Mr if ax == 0 else Mr.transpose(1, 0, 2)  # [slab, kdim, u]
            nz = sl.any(axis=1)                            # [slab, u]
            any_s = nz.any(axis=1)
            first = nz.argmax(axis=1)
            last = U - 1 - nz[:, ::-1].argmax(axis=1)
            lo = np.where(any_s, np.minimum(lo, first), lo)
            hi = np.where(any_s, np.maximum(hi, last), hi)
        # per-slab window widths (0 = no contribution, matmul skipped)
        wv = np.maximum(hi - lo + 1, 0).astype(np.int64)
        assert wv.max() <= U
        off = np.minimum(np.maximum(lo, 0), U - np.maximum(wv, 1))
        off = np.where(hi < 0, 0, off).astype(np.int64)
        # every psum column of the slot's 128-wide region must be written
        # by at least one slab matmul (unwritten psum is garbage)
        cover = np.zeros(U, bool)
        for s in range(S):
            cover[off[s]:off[s] + wv[s]] = True
        assert cover.all(), f"slot {j}: uncovered psum cols"
        axes.append(ax)
        widths.append(wv)
        offs.append(off)
    return Ms, np.array(axes), np.stack(widths), np.stack(offs)


def _layout(axes, widths, slots):
    """Slab-major packed column layout for one axis: returns (base[s] start
    col of slab s, cum[j][s] within-slab offset, total cols)."""
    base = np.zeros(S + 1, np.int64)
    cum = {j: np.zeros(S, np.int64) for j in slots}
    for s in range(S):
        c = 0
        for j in slots:
            cum[j][s] = c
            c += int(widths[j][s])
        base[s + 1] = base[s] + c
    return base, cum, int(base[S])


def _pack_core(Ms, axes, widths, offs, lay0, lay1, slots0, slots1, core):
    """Banded fp8 M streams for one core (variable per-slab widths)."""
    f8 = ml_dtypes.float8_e3m4
    base0, cum0, W0tot = lay0
    base1, cum1, W1tot = lay1
    m0 = np.zeros((S, W0tot), f8)
    m1 = np.zeros((S, W1tot), f8)
    for j in range(NSLOT):
        k = NCORES * j + core
        Mr = Ms[k].reshape(H, W, U)
        sl = Mr if axes[j] == 0 else Mr.transpose(1, 0, 2)   # [slab, kdim, u]
        m, base, cum = (m0, base0, cum0[j]) if axes[j] == 0 else (m1, base1, cum1[j])
        for s in range(S):
            w = int(widths[j][s])
            if w == 0:
                continue
            o = int(offs[j][s])
            c0 = int(base[s] + cum[s])
            m[:, c0:c0 + w] = sl[s][:, o:o + w].astype(f8)  # [kdim, w]
    return m0, m1


_COMPILED = {}


def _get_compiled(angles):
    key = hash(angles.tobytes())
    if key in _COMPILED:
        return _COMPILED[key]
    from contextlib import ExitStack
    import concourse.bacc as bacc
    import concourse.tile as tile
    import concourse.mybir as mybir

    Ms, axes, widths, offs = _schedule(angles)
    axis0_slots = [j for j in range(NSLOT) if axes[j] == 0]
    axis1_slots = [j for j in range(NSLOT) if axes[j] == 1]
    order = axis0_slots + axis1_slots        # psum/out column order
    pos = {j: p for p, j in enumerate(order)}
    banks = [order[b * 4:b * 4 + 4] for b in range(4)]
    bank_of = {j: b for b, bs in enumerate(banks) for j in bs}
    block_of = {j: bs.index(j) for bs in banks for j in bs}
    lay0 = _layout(axes, widths, axis0_slots)
    lay1 = _layout(axes, widths, axis1_slots)
    base0, cum0, W0tot = lay0
    base1, cum1, W1tot = lay1

    nc = bacc.Bacc("TRN2", target_bir_lowering=False, debug=False,
                   enable_asserts=False, num_devices=NCORES)
    bf16 = mybir.dt.bfloat16
    f8 = mybir.dt.float8e3
    f32 = mybir.dt.float32

    volA_d = nc.dram_tensor("volA", [S, H * D], bf16, kind="ExternalInput").ap()
    volB_d = nc.dram_tensor("volB", [S, H * D], bf16, kind="ExternalInput").ap()
    m0_d = nc.dram_tensor("m0", [S, W0tot], f8, kind="ExternalInput").ap()
    m1_d = nc.dram_tensor("m1", [S, W1tot], f8, kind="ExternalInput").ap()
    out_d = nc.dram_tensor("out", [V, NSLOT * U], bf16, kind="ExternalOutput").ap()

    with tile.TileContext(nc) as tc:
        with ExitStack() as ctx:
            sbuf = ctx.enter_context(tc.tile_pool(name="sbuf", bufs=1))
            psum = ctx.enter_context(tc.tile_pool(name="psum", bufs=1, space="PSUM"))

            volA_sb = sbuf.tile([S, H * D], bf16)
            volB_sb = sbuf.tile([S, H * D], bf16)
            m0_sb = sbuf.tile([S, W0tot], f8)
            m1_sb = sbuf.tile([S, W1tot], f8)
            out_sb = sbuf.tile([V, NSLOT * U], bf16)
            ps = [psum.tile([V, 512], f32, name=f"ps{b}") for b in range(4)]

            # Everything is SBUF-resident; chunked DMAs give fine-grained
            # deps so compute chases the stream frontier. Both hardware DGE
            # queues carry each phase's inputs in need order, greedily
            # byte-balanced. Chunk sizes are graded (small first) so the
            # first matmul starts early.
            bounds = [0, 4, 8, 16]
            while bounds[-1] < S:
                bounds.append(min(S, bounds[-1] + 16))
            mchunks = list(zip(bounds[:-1], bounds[1:]))
            vbounds = [0, 8, 16, 32, 64, 96, 128]
            vchunks = list(zip(vbounds[:-1], vbounds[1:]))

            items = []  # (need_slab, total_bytes, dst, src)
            for s0, s1 in vchunks:
                items.append((s0, (s1 - s0) * H * S * 2,
                              volA_sb[:, s0 * H:s1 * H], volA_d[:, s0 * H:s1 * H]))
            for s0, s1 in mchunks:
                c0, c1 = int(base0[s0]), int(base0[s1])
                items.append((s0, (c1 - c0) * S,
                              m0_sb[:, c0:c1], m0_d[:, c0:c1]))
            for s0, s1 in vchunks:
                items.append((S + s0, (s1 - s0) * H * S * 2,
                              volB_sb[:, s0 * H:s1 * H], volB_d[:, s0 * H:s1 * H]))
            for s0, s1 in mchunks:
                c0, c1 = int(base1[s0]), int(base1[s1])
                items.append((S + s0, (c1 - c0) * S,
                              m1_sb[:, c0:c1], m1_d[:, c0:c1]))
            items.sort(key=lambda it: it[0])
            qbytes = [0, 0]
            qeng = [nc.sync, nc.scalar]
            for _, nb, dst, src in items:
                qi = 0 if qbytes[0] <= qbytes[1] else 1
                qeng[qi].dma_start(dst, src)
                qbytes[qi] += nb

            # program-order first/last matmul per psum bank, for start/stop
            emitted = []
            for phase, slots in ((0, axis0_slots), (1, axis1_slots)):
                for s in range(S):
                    for j in slots:
                        if int(widths[j][s]) > 0:
                            emitted.append(bank_of[j])
            first_of_bank = {}
            last_of_bank = {}
            for i, b in enumerate(emitted):
                if b not in first_of_bank:
                    first_of_bank[b] = i
                last_of_bank[b] = i

            phase_banks = [sorted({bank_of[j] for j in axis0_slots}),
                           sorted({bank_of[j] for j in axis1_slots})]
            out_base = {}
            base = 0
            for b in range(4):
                out_base[b] = base
                base += len(banks[b]) * U

            i = 0
            for phase, slots in ((0, axis0_slots), (1, axis1_slots)):
                vol_sb = volA_sb if phase == 0 else volB_sb
                m_sb = m0_sb if phase == 0 else m1_sb
                baseT, cumT = (base0, cum0) if phase == 0 else (base1, cum1)
                for s in range(S):
                    lhsT = vol_sb[:, s * D:(s + 1) * D]
                    for j in slots:
                        w = int(widths[j][s])
                        if w == 0:
                            continue
                        b = bank_of[j]
                        col = block_of[j] * U + int(offs[j][s])
                        c0 = int(baseT[s] + cumT[j][s])
                        nc.tensor.matmul(
                            ps[b][:, col:col + w],
                            lhsT=lhsT,
                            rhs=m_sb[:, c0:c0 + w],
                            start=(i == first_of_bank[b]),
                            stop=(i == last_of_bank[b]),
                        )
                        i += 1
                # flush this phase's banks while the next phase computes.
                # Critically, the flush must NOT run on the sync/scalar
                # engines: it waits on the phase's matmuls, and would block
                # those engines from issuing the remaining stream DMAs. DVE
                # does the psum scale, the gpsimd SWDGE queue writes out.
                # phase 0: flush must avoid the stream-DMA engines (sync/
                # scalar) or it blocks their remaining issues. phase 1: the
                # streams are drained, so use both HW queues and both DVE +
                # scalar for a pipelined, parallel tail.
                if phase == 0:
                    mul_e = [nc.vector]
                    flush_q = [nc.gpsimd]
                    npiece = 2
                else:
                    mul_e = [nc.vector, nc.scalar]
                    flush_q = [nc.sync, nc.scalar]
                    npiece = 4
                nq = 0
                for bi, b in enumerate(phase_banks[phase]):
                    n = len(banks[b]) * U
                    step = n // npiece
                    me = mul_e[bi % len(mul_e)]  # one engine per psum bank
                    for o0 in range(0, n, step):
                        o1 = min(o0 + step, n)
                        if me is nc.scalar:
                            me.mul(out_sb[:, out_base[b] + o0:out_base[b] + o1],
                                   ps[b][:, o0:o1], float(DT))
                        else:
                            me.tensor_scalar_mul(
                                out_sb[:, out_base[b] + o0:out_base[b] + o1],
                                ps[b][:, o0:o1], float(DT))
                        flush_q[nq % len(flush_q)].dma_start(
                            out_d[:, out_base[b] + o0:out_base[b] + o1],
                            out_sb[:, out_base[b] + o0:out_base[b] + o1])
                        nq += 1

    nc.compile()
    meta = dict(nc=nc, Ms=Ms, axes=axes, widths=widths, offs=offs,
                lay0=lay0, lay1=lay1, slots0=axis0_slots, slots1=axis1_slots,
                order=order)
    _COMPILED[key] = meta
    return meta


def kernel(vol, angles):
    from concourse.bass_utils import run_bass_kernel_spmd

    vol = np.asarray(vol, dtype=np.float32)
    angles = np.asarray(angles, dtype=np.float32)
    meta = _get_compiled(angles)
    nc = meta["nc"]

    volA = vol[0, 0].reshape(S, H * D).astype(ml_dtypes.bfloat16)
    volB = np.ascontiguousarray(vol[0, 0].transpose(1, 0, 2)).reshape(
        S, H * D).astype(ml_dtypes.bfloat16)
    in_maps = []
    for c in range(NCORES):
        m0, m1 = _pack_core(meta["Ms"], meta["axes"], meta["widths"],
                            meta["offs"], meta["lay0"], meta["lay1"],
                            meta["slots0"], meta["slots1"], c)
        in_maps.append({"volA": volA, "volB": volB, "m0": m0, "m1": m1})

    res = run_bass_kernel_spmd(nc, in_maps, core_ids=list(range(NCORES)))
    global _LAST_RES
    _LAST_RES = res
    full = np.empty((1, 1, U, A, V), np.float32)
    for c, r in enumerate(res.results):
        rc = r["out"].astype(np.float32)    # [v, pos*128 + u] (bf16 on device)
        for p, j in enumerate(meta["order"]):
            k = NCORES * j + c
            full[0, 0, :, k, :] = rc[:, p * U:(p + 1) * U].T
    return full


# revision 32
# speedup vs baseline: 1.0320x; 1.0320x over previous
"""Joseph 3D projector on 8 TRN2 NeuronCores — banded-matmul version.

Formulation: for each angle a, out[u, v] = DT * sum_p M_a[p, u] * volT[p, v]
where p = y*128 + x and volT[p, v] = vol[0,0,x,y,v] (the reference's
z-interpolation is an exact identity for this geometry). M_a is ~1.5% dense:
for a fixed contraction slab (a y-line or x-line of the volume) the nonzero
u's lie in a narrow window of width ~128*min(|sin|,|cos|)+4. So instead of
streaming dense M (4MB/angle bf16, DMA-bound at ~212us), we stream only the
per-slab windows in fp8-e3m4 and issue one small matmul per (slab, angle)
accumulating at the window's column offset in PSUM (per-element has_written
semantics: first-touch overwrites, later touches accumulate).

Per angle the contraction axis is chosen as the driving axis (contract over
x with y-slabs when |sin|<=|cos|, else over y with x-slabs), against one of
two SBUF-resident volume layouts (volA=[x,(y,v)], volB=[y,(x,v)], bf16).

Sharding: 15 angle-slots; slot j on core c holds angle k = 8j + c, so all 8
cores share one SPMD program whose per-slot window geometry (width, per-slab
column offsets) covers the slot's 8 consecutive angles.

The stationary operand (the vol slab) is shared by all of a slab's matmuls;
redundant LDWEIGHTS are elided via InstMatmult.ldweights=False.
"""
import numpy as np
import ml_dtypes

D = H = W = 128
V = U = 128
A = 120
S = 128
NCORES = 8
NSLOT = A // NCORES  # 15
T = 0.5 * float(np.sqrt(((W - 1) * 1.0) ** 2 + ((H - 1) * 1.0) ** 2))
DT = 2.0 * T / S

ELIDE_LDW = False      # InstMatmult.ldweights=False proved inert on HW
MGROUP = 16            # slabs per M dma group
PREFETCH = 2           # dma groups emitted ahead of compute


def _build_M(cos_t, sin_t):
    """Dense M[p=(y*W+x), u] float32 for one angle."""
    u_phys = np.arange(U, dtype=np.float64) - (U - 1) / 2.0
    t = -T + (np.arange(S, dtype=np.float64) + 0.5) * DT
    x_idx = (-u_phys[None, :] * sin_t + t[:, None] * cos_t) + (W - 1) / 2.0
    y_idx = (u_phys[None, :] * cos_t + t[:, None] * sin_t) + (H - 1) / 2.0
    x0 = np.floor(x_idx).astype(np.int64)
    y0 = np.floor(y_idx).astype(np.int64)
    wx = x_idx - x0
    wy = y_idx - y0
    Mflat = np.zeros(H * W * U, np.float32)
    uu = np.broadcast_to(np.arange(U, dtype=np.int64)[None, :], (S, U))
    for dy, dx in ((0, 0), (0, 1), (1, 0), (1, 1)):
        yi = y0 + dy
        xi = x0 + dx
        w = (wy if dy else 1 - wy) * (wx if dx else 1 - wx)
        valid = (xi >= 0) & (xi <= W - 1) & (yi >= 0) & (yi <= H - 1)
        p = np.clip(yi, 0, H - 1) * W + np.clip(xi, 0, W - 1)
        flat = (p * U + uu)[valid]
        Mflat += np.bincount(flat, weights=w[valid].astype(np.float64),
                             minlength=H * W * U).astype(np.float32)
    return Mflat.reshape(H * W, U)


def _schedule(angles):
    """Slot geometry shared by all cores: per slot j (angles 8j..8j+7):
    contraction axis, window width, per-slab window offsets."""
    Ms = np.stack([_build_M(np.cos(np.float64(a)), np.sin(np.float64(a)))
                   for a in angles])              # [A, H*W, U]
    axes, widths, offs = [], [], []
    for j in range(NSLOT):
        ks = list(range(NCORES * j, NCORES * j + NCORES))
        th = [float(angles[k]) for k in ks]
        s_m = np.mean([abs(np.sin(t)) for t in th])
        c_m = np.mean([abs(np.cos(t)) for t in th])
        ax = 0 if s_m <= c_m else 1
        lo = np.full(S, U, np.int64)
        hi = np.full(S, -1, np.int64)
        for k in ks:
            Mr = Ms[k].reshape(H, W, U)
            sl = Mr if ax == 0 else Mr.transpose(1, 0, 2)  # [slab, kdim, u]
            nz = sl.any(axis=1)                            # [slab, u]
            any_s = nz.any(axis=1)
            first = nz.argmax(axis=1)
            last = U - 1 - nz[:, ::-1].argmax(axis=1)
            lo = np.where(any_s, np.minimum(lo, first), lo)
            hi = np.where(any_s, np.maximum(hi, last), hi)
        # per-slab window widths (0 = no contribution, matmul skipped)
        wv = np.maximum(hi - lo + 1, 0).astype(np.int64)
        assert wv.max() <= U
        off = np.minimum(np.maximum(lo, 0), U - np.maximum(wv, 1))
        off = np.where(hi < 0, 0, off).astype(np.int64)
        # every psum column of the slot's 128-wide region must be written
        # by at least one slab matmul (unwritten psum is garbage)
        cover = np.zeros(U, bool)
        for s in range(S):
            cover[off[s]:off[s] + wv[s]] = True
        assert cover.all(), f"slot {j}: uncovered psum cols"
        axes.append(ax)
        widths.append(wv)
        offs.append(off)
    return Ms, np.array(axes), np.stack(widths), np.stack(offs)


def _layout(axes, widths, slots):
    """Slab-major packed column layout for one axis: returns (base[s] start
    col of slab s, cum[j][s] within-slab offset, total cols)."""
    base = np.zeros(S + 1, np.int64)
    cum = {j: np.zeros(S, np.int64) for j in slots}
    for s in range(S):
        c = 0
        for j in slots:
            cum[j][s] = c
            c += int(widths[j][s])
        base[s + 1] = base[s] + c
    return base, cum, int(base[S])


def _pack_core(Ms, axes, widths, offs, lay0, lay1, slots0, slots1, core):
    """Banded fp8 M streams for one core (variable per-slab widths)."""
    f8 = ml_dtypes.float8_e3m4
    base0, cum0, W0tot = lay0
    base1, cum1, W1tot = lay1
    m0 = np.zeros((S, W0tot), f8)
    m1 = np.zeros((S, W1tot), f8)
    for j in range(NSLOT):
        k = NCORES * j + core
        Mr = Ms[k].reshape(H, W, U)
        sl = Mr if axes[j] == 0 else Mr.transpose(1, 0, 2)   # [slab, kdim, u]
        m, base, cum = (m0, base0, cum0[j]) if axes[j] == 0 else (m1, base1, cum1[j])
        for s in range(S):
            w = int(widths[j][s])
            if w == 0:
                continue
            o = int(offs[j][s])
            c0 = int(base[s] + cum[s])
            m[:, c0:c0 + w] = sl[s][:, o:o + w].astype(f8)  # [kdim, w]
    return m0, m1


_COMPILED = {}


def _get_compiled(angles):
    key = hash(angles.tobytes())
    if key in _COMPILED:
        return _COMPILED[key]
    from contextlib import ExitStack
    import concourse.bacc as bacc
    import concourse.tile as tile
    import concourse.mybir as mybir

    Ms, axes, widths, offs = _schedule(angles)
    axis0_slots = [j for j in range(NSLOT) if axes[j] == 0]
    axis1_slots = [j for j in range(NSLOT) if axes[j] == 1]
    order = axis0_slots + axis1_slots        # psum/out column order
    pos = {j: p for p, j in enumerate(order)}
    banks = [order[b * 4:b * 4 + 4] for b in range(4)]
    bank_of = {j: b for b, bs in enumerate(banks) for j in bs}
    block_of = {j: bs.index(j) for bs in banks for j in bs}
    lay0 = _layout(axes, widths, axis0_slots)
    lay1 = _layout(axes, widths, axis1_slots)
    base0, cum0, W0tot = lay0
    base1, cum1, W1tot = lay1

    nc = bacc.Bacc("TRN2", target_bir_lowering=False, debug=False,
                   enable_asserts=False, num_devices=NCORES)
    bf16 = mybir.dt.bfloat16
    f8 = mybir.dt.float8e3
    f32 = mybir.dt.float32

    volA_d = nc.dram_tensor("volA", [S, H * D], bf16, kind="ExternalInput").ap()
    volB_d = nc.dram_tensor("volB", [S, H * D], bf16, kind="ExternalInput").ap()
    m0_d = nc.dram_tensor("m0", [S, W0tot], f8, kind="ExternalInput").ap()
    m1_d = nc.dram_tensor("m1", [S, W1tot], f8, kind="ExternalInput").ap()
    out_d = nc.dram_tensor("out", [V, NSLOT * U], f32, kind="ExternalOutput").ap()

    with tile.TileContext(nc) as tc:
        with ExitStack() as ctx:
            sbuf = ctx.enter_context(tc.tile_pool(name="sbuf", bufs=1))
            psum = ctx.enter_context(tc.tile_pool(name="psum", bufs=1, space="PSUM"))

            volA_sb = sbuf.tile([S, H * D], bf16)
            volB_sb = sbuf.tile([S, H * D], bf16)
            m0_sb = sbuf.tile([S, W0tot], f8)
            m1_sb = sbuf.tile([S, W1tot], f8)
            out_sb = sbuf.tile([V, NSLOT * U], f32)
            ps = [psum.tile([V, 512], f32, name=f"ps{b}") for b in range(4)]

            # Everything is SBUF-resident; chunked DMAs give fine-grained
            # deps so compute chases the stream frontier. Both hardware DGE
            # queues carry each phase's inputs in need order, greedily
            # byte-balanced. Chunk sizes are graded (small first) so the
            # first matmul starts early.
            bounds = [0, 4, 8, 16]
            while bounds[-1] < S:
                bounds.append(min(S, bounds[-1] + 16))
            mchunks = list(zip(bounds[:-1], bounds[1:]))
            vbounds = [0, 8, 16, 32, 64, 96, 128]
            vchunks = list(zip(vbounds[:-1], vbounds[1:]))

            items = []  # (need_slab, total_bytes, dst, src)
            for s0, s1 in vchunks:
                items.append((s0, (s1 - s0) * H * S * 2,
                              volA_sb[:, s0 * H:s1 * H], volA_d[:, s0 * H:s1 * H]))
            for s0, s1 in mchunks:
                c0, c1 = int(base0[s0]), int(base0[s1])
                items.append((s0, (c1 - c0) * S,
                              m0_sb[:, c0:c1], m0_d[:, c0:c1]))
            for s0, s1 in vchunks:
                items.append((S + s0, (s1 - s0) * H * S * 2,
                              volB_sb[:, s0 * H:s1 * H], volB_d[:, s0 * H:s1 * H]))
            for s0, s1 in mchunks:
                c0, c1 = int(base1[s0]), int(base1[s1])
                items.append((S + s0, (c1 - c0) * S,
                              m1_sb[:, c0:c1], m1_d[:, c0:c1]))
            items.sort(key=lambda it: it[0])
            qbytes = [0, 0]
            qeng = [nc.sync, nc.scalar]
            for _, nb, dst, src in items:
                qi = 0 if qbytes[0] <= qbytes[1] else 1
                qeng[qi].dma_start(dst, src)
                qbytes[qi] += nb

            # program-order first/last matmul per psum bank, for start/stop
            emitted = []
            for phase, slots in ((0, axis0_slots), (1, axis1_slots)):
                for s in range(S):
                    for j in slots:
                        if int(widths[j][s]) > 0:
                            emitted.append(bank_of[j])
            first_of_bank = {}
            last_of_bank = {}
            for i, b in enumerate(emitted):
                if b not in first_of_bank:
                    first_of_bank[b] = i
                last_of_bank[b] = i

            phase_banks = [sorted({bank_of[j] for j in axis0_slots}),
                           sorted({bank_of[j] for j in axis1_slots})]
            out_base = {}
            base = 0
            for b in range(4):
                out_base[b] = base
                base += len(banks[b]) * U

            i = 0
            for phase, slots in ((0, axis0_slots), (1, axis1_slots)):
                vol_sb = volA_sb if phase == 0 else volB_sb
                m_sb = m0_sb if phase == 0 else m1_sb
                baseT, cumT = (base0, cum0) if phase == 0 else (base1, cum1)
                for s in range(S):
                    lhsT = vol_sb[:, s * D:(s + 1) * D]
                    for j in slots:
                        w = int(widths[j][s])
                        if w == 0:
                            continue
                        b = bank_of[j]
                        col = block_of[j] * U + int(offs[j][s])
                        c0 = int(baseT[s] + cumT[j][s])
                        nc.tensor.matmul(
                            ps[b][:, col:col + w],
                            lhsT=lhsT,
                            rhs=m_sb[:, c0:c0 + w],
                            start=(i == first_of_bank[b]),
                            stop=(i == last_of_bank[b]),
                        )
                        i += 1
                # flush this phase's banks while the next phase computes.
                # Critically, the flush must NOT run on the sync/scalar
                # engines: it waits on the phase's matmuls, and would block
                # those engines from issuing the remaining stream DMAs. DVE
                # does the psum scale, the gpsimd SWDGE queue writes out.
                # Critically, the phase-0 flush must NOT run on the sync/
                # scalar engines: it waits on the phase's matmuls, and would
                # block those engines from issuing the remaining stream DMAs.
                # DVE does the psum scale; phase-0 out rides the gpsimd SWDGE
                # queue, the final flush uses the (drained) HW queues.
                flush_q = [nc.gpsimd] if phase == 0 else [nc.sync, nc.scalar]
                nq = 0
                for b in phase_banks[phase]:
                    n = len(banks[b]) * U
                    h = n // 2
                    for o0, o1 in ((0, h), (h, n)):
                        nc.vector.tensor_scalar_mul(
                            out_sb[:, out_base[b] + o0:out_base[b] + o1],
                            ps[b][:, o0:o1], float(DT))
                        flush_q[nq % len(flush_q)].dma_start(
                            out_d[:, out_base[b] + o0:out_base[b] + o1],
                            out_sb[:, out_base[b] + o0:out_base[b] + o1])
                        nq += 1

    nc.compile()
    meta = dict(nc=nc, Ms=Ms, axes=axes, widths=widths, offs=offs,
                lay0=lay0, lay1=lay1, slots0=axis0_slots, slots1=axis1_slots,
                order=order)
    _COMPILED[key] = meta
    return meta


def kernel(vol, angles):
    from concourse.bass_utils import run_bass_kernel_spmd

    vol = np.asarray(vol, dtype=np.float32)
    angles = np.asarray(angles, dtype=np.float32)
    meta = _get_compiled(angles)
    nc = meta["nc"]

    volA = vol[0, 0].reshape(S, H * D).astype(ml_dtypes.bfloat16)
    volB = np.ascontiguousarray(vol[0, 0].transpose(1, 0, 2)).reshape(
        S, H * D).astype(ml_dtypes.bfloat16)
    in_maps = []
    for c in range(NCORES):
        m0, m1 = _pack_core(meta["Ms"], meta["axes"], meta["widths"],
                            meta["offs"], meta["lay0"], meta["lay1"],
                            meta["slots0"], meta["slots1"], c)
        in_maps.append({"volA": volA, "volB": volB, "m0": m0, "m1": m1})

    res = run_bass_kernel_spmd(nc, in_maps, core_ids=list(range(NCORES)))
    global _LAST_RES
    _LAST_RES = res
    full = np.empty((1, 1, U, A, V), np.float32)
    for c, r in enumerate(res.results):
        rc = r["out"].astype(np.float32)    # [v, pos*128 + u] (bf16 on device)
        for p, j in enumerate(meta["order"]):
            k = NCORES * j + c
            full[0, 0, :, k, :] = rc[:, p * U:(p + 1) * U].T
    return full
